# revision 1
# baseline (speedup 1.0000x reference)
"""Trainium2 Bass kernel for nn_CalcImpute (retrieval KNN imputation).

Computes, per row r of dist_pot_donors [8192, 32768]:
  - the 16 smallest distances (ties broken by lowest column index, matching
    jax.lax.top_k on the negated matrix),
  - inverse-distance weights (with sklearn-style handling of exact-zero
    distances: rows containing a zero distance use {0,1} weights),
  - masked weighted mean of fit_X_col at the selected indices.

Sharding: rows are data-parallel across 8 NeuronCores (1024 rows each);
fit_X_col / mask_fit_X_col are replicated (combined into one small table).

Device algorithm (per core, per block of 128 rows on partitions):
  1. Stream the block's [128, 32768] distances in 8 panels; DVE
     tensor_reduce(min, negate=True) produces negated minima of 2048
     16-element column chunks.
  2. Select the top-16 chunks per row by (chunk_min asc, chunk_idx asc)
     via fp32 max8 + max_index (first-match = lowest index) +
     match_replace + max8/max_index.  A row's true top-16 elements
     provably live in these 16 chunks for ANY input.
  3. Indirect-DMA regather of the 16 winning chunks per row (and of the
     combined xm/mb donor table at the same chunk ids).
  4. Dense exact selection on the gathered [128, 256] values: boundary
     value V = 16th smallest; elements < V always selected; elements == V
     selected in ascending global-index order until 16 — implemented
     arithmetically (no data-dependent branches).
  5. Weights 1/d with zero-distance row fixup, masked weighted mean,
     result DMA'd out.

Assumptions: no NaNs in the distance matrix (inputs are uniform [0,1)),
n_neighbors == 16.  Everything else (ties, exact zeros, all-masked rows)
is handled exactly.
"""

from contextlib import ExitStack

import numpy as np

import concourse.bacc as bacc
import concourse.bass as bass
import concourse.mybir as mybir
import concourse.tile as tile
from concourse import bass_utils

F32 = mybir.dt.float32
I32 = mybir.dt.int32
U32 = mybir.dt.uint32
ALU = mybir.AluOpType
ACTF = mybir.ActivationFunctionType

R_FULL = 8192          # total rows
D = 32768              # donors (columns)
K = 16                 # n_neighbors
N_CORES = 8
RPC = R_FULL // N_CORES  # rows per core (1024)
P = 128                # partitions
NB = RPC // P          # row blocks per core (8)
NPAN = 4               # column panels per block
W = D // NPAN          # panel width (8192)
CH = 16                # chunk length
NCHUNK = D // CH       # chunks per row (2048)
CPP = W // CH          # chunks per panel (256)
GW = K * CH            # gathered width per row (256)
NEG_SENT = -3.0e38


def build_module(rep: int = 1, stages: str = "full", small_input: bool = False):
    """Build the Bass module. rep>1 wraps the compute body in an on-device
    For_i loop (for timing: marginal cost per rep = true kernel time, fixed
    overheads like the 1 GiB axon upload cancel).

    stages: "full" | "l1" | "l2" | "gather" | "l3ng" — ablation levels for
    timing attribution ("l3ng" = level-3 math with memset instead of
    gathers).

    small_input: timing-only — d input is a single 16 MiB block read 8x per
    rep (64x smaller upload; faithful HBM/compute behavior)."""
    do_l2 = stages in ("l2", "gather", "l3ng", "full")
    do_gather = stages in ("gather", "full")
    do_l3 = stages in ("l3ng", "full")
    nc = bacc.Bacc("TRN2", target_bir_lowering=False, debug=False)

    d_rows = P if small_input else RPC
    d_dram = nc.dram_tensor("d", (d_rows, D), F32, kind="ExternalInput")
    dxm_dram = nc.dram_tensor("dxm", (NCHUNK, 2 * CH), F32, kind="ExternalInput")
    res_dram = nc.dram_tensor("res", (RPC, 1), F32, kind="ExternalOutput")

    # chunk-granular view of the shard for the indirect regather
    d_chunks = d_dram.ap().rearrange("r (c t) -> (r c) t", t=CH)

    with tile.TileContext(nc) as tc:
        with ExitStack() as ctx:
            const = ctx.enter_context(tc.tile_pool(name="const", bufs=1))
            dpool = ctx.enter_context(tc.tile_pool(name="dpool", bufs=3))
            mpool = ctx.enter_context(tc.tile_pool(name="mpool", bufs=2))
            spool = ctx.enter_context(tc.tile_pool(name="spool", bufs=3))

            # ---- constants ----
            iota_t_i = const.tile([P, GW], I32, tag="iota_t_i")
            nc.gpsimd.iota(iota_t_i[:].rearrange("p (c t) -> p c t", t=CH),
                           pattern=[[0, K], [1, CH]], base=0,
                           channel_multiplier=0)
            iota_t_f = const.tile([P, GW], F32, tag="iota_t_f")
            nc.vector.tensor_copy(iota_t_f[:], iota_t_i[:])

            iota16_i = const.tile([P, K], I32, tag="iota16_i")
            nc.gpsimd.iota(iota16_i[:], pattern=[[1, K]], base=0,
                           channel_multiplier=0)
            iota16_f = const.tile([P, K], F32, tag="iota16_f")
            nc.vector.tensor_copy(iota16_f[:], iota16_i[:])

            # per-partition row index (within block) * NCHUNK
            rowb_i = const.tile([P, 1], I32, tag="rowb_i")
            nc.gpsimd.iota(rowb_i[:], pattern=[[0, 1]], base=0,
                           channel_multiplier=NCHUNK)
            rowb_f = const.tile([P, 1], F32, tag="rowb_f")
            nc.vector.tensor_copy(rowb_f[:], rowb_i[:])

            loop_ctx = tc.For_i(0, rep, 1) if rep > 1 else None
            if loop_ctx is not None:
                loop_ctx.__enter__()
            if True:
                for b in range(NB):
                    # ---------- level 1: negated chunk minima ----------
                    negmins = mpool.tile([P, NCHUNK], F32, tag="negmins")
                    for p in range(NPAN):
                        dt = dpool.tile([P, W], F32, tag="dt")
                        rb = 0 if small_input else b * P
                        nc.sync.dma_start(
                            dt[:], d_dram.ap()[rb:rb + P,
                                               p * W:(p + 1) * W])
                        nc.vector.tensor_reduce(
                            negmins[:, p * CPP:(p + 1) * CPP],
                            dt[:].rearrange("p (c t) -> p c t", t=CH),
                            axis=mybir.AxisListType.X, op=ALU.min, negate=True)

                    if not do_l2:
                        res_b = spool.tile([P, 1], F32, tag="res_b")
                        nc.vector.tensor_scalar(res_b[:], negmins[:, 0:1],
                                                1.0, None, op0=ALU.mult)
                        nc.sync.dma_start(
                            res_dram.ap()[b * P:(b + 1) * P, :], res_b[:])
                        continue

                    # ---------- level 2: top-16 chunks per row ----------
                    t8a = spool.tile([P, 8], F32, tag="t8a")
                    nc.vector.max(t8a[:], negmins[:])
                    pos_a = spool.tile([P, 8], U32, tag="pos_a")
                    nc.vector.max_index(pos_a[:], t8a[:], negmins[:])
                    negmins2 = mpool.tile([P, NCHUNK], F32, tag="negmins2")
                    nc.vector.match_replace(negmins2[:], t8a[:], negmins[:],
                                            NEG_SENT)
                    t8b = spool.tile([P, 8], F32, tag="t8b")
                    nc.vector.max(t8b[:], negmins2[:])
                    pos_b = spool.tile([P, 8], U32, tag="pos_b")
                    nc.vector.max_index(pos_b[:], t8b[:], negmins2[:])

                    cidx_f = spool.tile([P, K], F32, tag="cidx_f")
                    nc.vector.tensor_copy(cidx_f[:, 0:8], pos_a[:])
                    nc.vector.tensor_copy(cidx_f[:, 8:16], pos_b[:])

                    # chunk ids -> shard-global gather indices (f32 then i32)
                    rowbase_b = spool.tile([P, 1], F32, tag="rowbase_b")
                    rbase = 0.0 if small_input else float(b * P * NCHUNK)
                    nc.vector.tensor_scalar(rowbase_b[:], rowb_f[:],
                                            rbase, None, op0=ALU.add)
                    gidx_f = spool.tile([P, K], F32, tag="gidx_f")
                    nc.vector.tensor_scalar(gidx_f[:], cidx_f[:],
                                            rowbase_b[:], None, op0=ALU.add)
                    gidx_i = spool.tile([P, K], I32, tag="gidx_i")
                    nc.vector.tensor_copy(gidx_i[:], gidx_f[:])
                    cidx_i = spool.tile([P, K], I32, tag="cidx_i")
                    nc.vector.tensor_copy(cidx_i[:], cidx_f[:])

                    # ---------- level 3: regather + exact selection ----------
                    G = spool.tile([P, K, CH], F32, tag="G")
                    XB = spool.tile([P, K, 2 * CH], F32, tag="XB")
                    if do_gather:
                        for j in range(K):
                            nc.gpsimd.indirect_dma_start(
                                G[:, j, :], None, d_chunks,
                                bass.IndirectOffsetOnAxis(
                                    ap=gidx_i[:, j:j + 1], axis=0))
                            nc.gpsimd.indirect_dma_start(
                                XB[:, j, :], None, dxm_dram.ap(),
                                bass.IndirectOffsetOnAxis(
                                    ap=cidx_i[:, j:j + 1], axis=0))
                    elif do_l3:
                        nc.vector.memset(G[:], 0.5)
                        nc.vector.memset(XB[:], 1.0)
                    if not do_l3:
                        res_b = spool.tile([P, 1], F32, tag="res_b")
                        nc.vector.tensor_scalar(res_b[:], cidx_f[:, 0:1],
                                                1.0, None, op0=ALU.mult)
                        nc.sync.dma_start(
                            res_dram.ap()[b * P:(b + 1) * P, :], res_b[:])
                        continue
                    Gf = G[:].rearrange("p c t -> p (c t)")

                    # global element index X per gathered slot
                    X = spool.tile([P, GW], F32, tag="X")
                    cid_b = cidx_f[:].unsqueeze(-1).broadcast_to((P, K, CH))
                    nc.vector.scalar_tensor_tensor(
                        X[:].rearrange("p (c t) -> p c t", t=CH),
                        cid_b, float(CH),
                        iota_t_f[:].rearrange("p (c t) -> p c t", t=CH),
                        op0=ALU.mult, op1=ALU.add)

                    # 16th smallest value V
                    negG = spool.tile([P, GW], F32, tag="negG")
                    nc.scalar.activation(negG[:], Gf, ACTF.Copy, scale=-1.0)
                    g8a = spool.tile([P, 8], F32, tag="g8a")
                    nc.vector.max(g8a[:], negG[:])
                    negG2 = spool.tile([P, GW], F32, tag="negG2")
                    nc.vector.match_replace(negG2[:], g8a[:], negG[:], NEG_SENT)
                    g8b = spool.tile([P, 8], F32, tag="g8b")
                    nc.vector.max(g8b[:], negG2[:])
                    V = spool.tile([P, 1], F32, tag="V")
                    nc.vector.tensor_scalar(V[:], g8b[:, 7:8], -1.0, None,
                                            op0=ALU.mult)

                    # strictly-below mask + count
                    maskLT = spool.tile([P, GW], F32, tag="maskLT")
                    q = spool.tile([P, 1], F32, tag="q")
                    nc.vector.tensor_scalar(maskLT[:], Gf, V[:], None,
                                            op0=ALU.is_lt, op1=ALU.add,
                                            accum_out=q[:])
                    maskEQ = spool.tile([P, GW], F32, tag="maskEQ")
                    nc.gpsimd.tensor_scalar(maskEQ[:], Gf, V[:], None,
                                            op0=ALU.is_equal)

                    # ascending global indices of ==V elements (up to 16)
                    t1 = spool.tile([P, GW], F32, tag="t1")
                    nc.gpsimd.tensor_scalar(t1[:], X[:], -1.0, 65536.0,
                                            op0=ALU.mult, op1=ALU.add)
                    t2 = spool.tile([P, GW], F32, tag="t2")
                    nc.vector.tensor_tensor(t2[:], t1[:], maskEQ[:],
                                            op=ALU.mult)
                    e8a = spool.tile([P, 8], F32, tag="e8a")
                    nc.vector.max(e8a[:], t2[:])
                    t2b = spool.tile([P, GW], F32, tag="t2b")
                    nc.vector.match_replace(t2b[:], e8a[:], t2[:], NEG_SENT)
                    e8b = spool.tile([P, 8], F32, tag="e8b")
                    nc.vector.max(e8b[:], t2b[:])
                    xs = spool.tile([P, K], F32, tag="xs")
                    nc.vector.tensor_scalar(xs[:, 0:8], e8a[:], -1.0, 65536.0,
                                            op0=ALU.mult, op1=ALU.add)
                    nc.vector.tensor_scalar(xs[:, 8:16], e8b[:], -1.0, 65536.0,
                                            op0=ALU.mult, op1=ALU.add)

                    # admit the (16 - q) lowest-index ==V elements
                    rq = spool.tile([P, 1], F32, tag="rq")
                    nc.vector.tensor_scalar(rq[:], q[:], -1.0, float(K),
                                            op0=ALU.mult, op1=ALU.add)
                    t3 = spool.tile([P, K], F32, tag="t3")
                    nc.vector.tensor_scalar(t3[:], iota16_f[:], rq[:], None,
                                            op0=ALU.is_lt)
                    t4 = spool.tile([P, K], F32, tag="t4")
                    nc.vector.tensor_tensor(t4[:], t3[:], xs[:], op=ALU.mult)
                    xthr = spool.tile([P, 1], F32, tag="xthr")
                    nc.vector.tensor_reduce(xthr[:], t4[:],
                                            axis=mybir.AxisListType.X,
                                            op=ALU.max)
                    t5 = spool.tile([P, GW], F32, tag="t5")
                    nc.gpsimd.tensor_scalar(t5[:], X[:], xthr[:], None,
                                            op0=ALU.is_le)
                    selEQ = spool.tile([P, GW], F32, tag="selEQ")
                    nc.vector.tensor_tensor(selEQ[:], t5[:], maskEQ[:],
                                            op=ALU.mult)
                    sel = spool.tile([P, GW], F32, tag="sel")
                    nc.vector.tensor_tensor(sel[:], maskLT[:], selEQ[:],
                                            op=ALU.add)

                    # weights 1/d with zero-distance row fixup
                    z = spool.tile([P, GW], F32, tag="z")
                    nc.gpsimd.tensor_scalar(z[:], Gf, 0.0, None,
                                            op0=ALU.is_equal)
                    zr = spool.tile([P, 1], F32, tag="zr")
                    nc.vector.tensor_reduce(zr[:], z[:],
                                            axis=mybir.AxisListType.X,
                                            op=ALU.max)
                    Gsafe = spool.tile([P, GW], F32, tag="Gsafe")
                    nc.gpsimd.tensor_scalar(Gsafe[:], Gf, zr[:], None,
                                            op0=ALU.add)
                    wb = spool.tile([P, GW], F32, tag="wb")
                    nc.vector.reciprocal(wb[:], Gsafe[:])
                    zc = spool.tile([P, 1], F32, tag="zc")
                    nc.vector.tensor_scalar(zc[:], zr[:], -1.0, 1.0,
                                            op0=ALU.mult, op1=ALU.add)
                    t6 = spool.tile([P, GW], F32, tag="t6")
                    nc.scalar.activation(t6[:], wb[:], ACTF.Copy, scale=zc[:])
                    t7 = spool.tile([P, GW], F32, tag="t7")
                    nc.gpsimd.tensor_scalar(t7[:], z[:], zr[:], None,
                                            op0=ALU.mult)
                    wfin = spool.tile([P, GW], F32, tag="wfin")
                    nc.vector.tensor_tensor(wfin[:], t6[:], t7[:], op=ALU.add)
                    wsel = spool.tile([P, GW], F32, tag="wsel")
                    nc.vector.tensor_tensor(wsel[:], wfin[:], sel[:],
                                            op=ALU.mult)

                    # masked weighted mean
                    junk1 = spool.tile([P, GW], F32, tag="junk1")
                    num = spool.tile([P, 1], F32, tag="num")
                    nc.vector.scalar_tensor_tensor(
                        junk1[:].rearrange("p (c t) -> p c t", t=CH),
                        wsel[:].rearrange("p (c t) -> p c t", t=CH), 1.0,
                        XB[:, :, 0:CH],
                        op0=ALU.mult, op1=ALU.mult, accum_out=num[:])
                    junk2 = spool.tile([P, GW], F32, tag="junk2")
                    den = spool.tile([P, 1], F32, tag="den")
                    nc.vector.scalar_tensor_tensor(
                        junk2[:].rearrange("p (c t) -> p c t", t=CH),
                        wsel[:].rearrange("p (c t) -> p c t", t=CH), 1.0,
                        XB[:, :, CH:2 * CH],
                        op0=ALU.mult, op1=ALU.mult, accum_out=den[:])

                    den0 = spool.tile([P, 1], F32, tag="den0")
                    nc.vector.scalar_tensor_tensor(
                        den0[:], den[:], 0.0, den[:],
                        op0=ALU.is_equal, op1=ALU.add)
                    rden = spool.tile([P, 1], F32, tag="rden")
                    nc.vector.reciprocal(rden[:], den0[:])
                    res_b = spool.tile([P, 1], F32, tag="res_b")
                    nc.vector.tensor_tensor(res_b[:], num[:], rden[:],
                                            op=ALU.mult)
                    nc.sync.dma_start(res_dram.ap()[b * P:(b + 1) * P, :],
                                      res_b[:])
            if loop_ctx is not None:
                loop_ctx.__exit__(None, None, None)

    nc.compile()
    return nc


_module_cache = {}


def _get_module(rep: int = 1):
    if rep not in _module_cache:
        _module_cache[rep] = build_module(rep)
    return _module_cache[rep]


def _prep_inputs(dist_pot_donors, fit_X_col, mask_fit_X_col):
    d = np.ascontiguousarray(np.asarray(dist_pot_donors, dtype=np.float32))
    assert d.shape == (R_FULL, D), d.shape
    x = np.asarray(fit_X_col, dtype=np.float32).reshape(D)
    m = np.asarray(mask_fit_X_col).reshape(D)
    mb = (1 - m).astype(np.float32)
    xm = (x * mb).astype(np.float32)
    dxm = np.concatenate([xm.reshape(NCHUNK, CH), mb.reshape(NCHUNK, CH)],
                         axis=1)
    dxm = np.ascontiguousarray(dxm)
    in_maps = [{"d": d[c * RPC:(c + 1) * RPC], "dxm": dxm}
               for c in range(N_CORES)]
    return in_maps


def kernel(dist_pot_donors, n_neighbors, fit_X_col, mask_fit_X_col):
    assert int(n_neighbors) == K, n_neighbors
    in_maps = _prep_inputs(dist_pot_donors, fit_X_col, mask_fit_X_col)
    nc = _get_module()
    r = bass_utils.run_bass_kernel_spmd(nc, in_maps,
                                        core_ids=list(range(N_CORES)))
    out = np.concatenate([r.results[c]["res"].reshape(RPC)
                          for c in range(N_CORES)])
    return out.astype(np.float32)

